# revision 40
# baseline (speedup 1.0000x reference)
"""Trainium2 Bass kernel: single-layer transformer encoder block.

reference:  LayerNorm -> fused QKV proj -> full softmax attention -> FC+LeakyReLU
inputs:     x [8, 2048, 512] f32 (+ LN gamma/beta, W_qkv [512,1536], W_fc [512,512], b_fc)

Sharding: pure data-parallel over batch -- each of the 8 NeuronCores gets one
batch element [S=2048, D=512]; weights are replicated. No collectives.

Per-core pipeline (matmuls in bf16 with f32 PSUM accumulation; LN stats f32):

  phase A  per 128-row tile: DMA x, bn_stats/bn_aggr, rstd = exp(-0.5*ln(var+eps)),
           xn = (x-mean)*rstd  (gamma/beta folded into W/bias on host), cast bf16,
           PE-transpose into feature-major xnT [d, s]
  phase B  kT[e,s] (weights stationary), V[t,d] (activations stationary)
  phase C  per 512-column query chunk:
             qT[e,s] for the chunk
             per 128-row key tile:  S^T = kT^T qT  (4 MMs), E = exp(S^T*scale)
                                    Z += ones^T E  (column sums, broadcast)
                                    O^T[dt] += V^T E
             oT = O^T * reciprocal(Z)
             y = LeakyReLU(oT^T W_fc + b_fc)  (bias via K=1 matmul) -> DMA out

Softmax skips max-subtraction: logits are ~N(0,1) after the 1/sqrt(D) scale
(bounded by a few sigma), so exp() is far from overflow in f32/bf16.
"""

import numpy as np
import ml_dtypes

import concourse.bass as bass
import concourse.mybir as mybir
import concourse.tile as tile
from concourse import bacc
from concourse.bass_utils import run_bass_kernel_spmd
from concourse.masks import make_identity
from concourse.tile_rust import add_dep_helper

F32 = mybir.dt.float32
BF16 = mybir.dt.bfloat16
FP8 = mybir.dt.float8e4
F32R = mybir.dt.float32r
AF = mybir.ActivationFunctionType
OP = mybir.AluOpType

D = 512
E3 = 3 * D
ND = D // 128  # 4 feature tiles
LN_EPS = 1e-5
SLOPE = 0.01
N_CORES = 8


def build_nc(S=2048, has_bv=False):
    NT = S // 128   # key/seq tiles
    NSC = S // 512  # query chunks
    SM_SCALE = float(D) ** -0.5

    nc = bacc.Bacc("TRN2", target_bir_lowering=False, debug=False)
    x_d = nc.dram_tensor("x", [S, D], F32, kind="ExternalInput")
    wqkv_d = nc.dram_tensor("wqkv", [128, ND, E3], BF16, kind="ExternalInput")
    wfc_d = nc.dram_tensor("wfc", [128, ND, D], BF16, kind="ExternalInput")
    bqkv_d = nc.dram_tensor("bqkv", [128, 3 * ND], F32, kind="ExternalInput")
    bfc_d = nc.dram_tensor("bfc", [1, D], F32, kind="ExternalInput")
    out_d = nc.dram_tensor("out", [S, D], F32, kind="ExternalOutput")

    with tile.TileContext(nc) as tc:
        with (
            tc.tile_pool(name="consts", bufs=1) as consts,
            tc.tile_pool(name="persist", bufs=1) as persist,
            tc.tile_pool(name="ln", bufs=6) as lnp,
            tc.tile_pool(name="eb", bufs=2) as ebp,
            tc.tile_pool(name="zb", bufs=2) as zbp,
            tc.tile_pool(name="esb", bufs=2) as esb,
            tc.tile_pool(name="yb", bufs=3) as ybp,
            tc.tile_pool(name="psA", bufs=2, space=bass.MemorySpace.PSUM) as psA,
            tc.tile_pool(name="psO", bufs=2, space=bass.MemorySpace.PSUM) as psO,
            tc.tile_pool(name="psFC", bufs=2, space=bass.MemorySpace.PSUM) as psFC,
        ):
            # ---- constants ----
            # (weight DMAs are emitted after the first x chunks below so the
            # LN pipeline starts as early as possible)
            wqkv_sb = consts.tile([128, ND, E3], BF16)
            wfc_sb = consts.tile([128, ND, D], BF16)
            bqkv_sb = consts.tile([128, 3 * ND], F32)
            bfc_sb = consts.tile([128, D], F32)
            ident = consts.tile([128, 128], BF16)
            make_identity(nc, ident)
            ones_f = consts.tile([128, 128], F32)
            nc.vector.memset(ones_f, 1.0)
            ones_r = consts.tile([128, 128], F32R)
            nc.vector.tensor_copy(out=ones_r, in_=ones_f)
            eps_sb = consts.tile([128, 1], F32)
            nc.vector.memset(eps_sb, LN_EPS)
            zero_sb = consts.tile([128, 1], F32)
            nc.vector.memset(zero_sb, 0.0)

            # ---- persistent activations ----
            xnT = persist.tile([128, ND, S], BF16)   # xn^T: [d_in_tile, d_tile, s]
            qT = persist.tile([128, ND, S], BF16)    # q^T: [e_in_tile, e_tile, s]
            kT = persist.tile([128, ND, S], BF16)
            vv = persist.tile([128, NT, D], BF16)    # V: [t_in_tile, t_tile, d]

            # ---- phase A: LN + transpose + V + k/q, pipelined per tile ----
            # x arrives in growing chunks (1,1,2,4,...) spread over the three
            # DMA-capable queues so tile 0 lands early and the per-tile chain
            # (bn stats -> sqrt -> 1/x -> normalize -> transpose) starts ~10us
            # in. rstd = 1/sqrt(var+eps) uses ACT Sqrt (its table set also
            # holds Identity; the Exp set loads once, later) + fast DVE recip.
            x_r = x_d.rearrange("(t p) d -> p t d", p=128)
            x_tiles = persist.tile([128, NT, D], F32, name="x_tiles")
            # wqkv first on sync (it gates the first V matmul group), followed
            # by the tail third of x; the head of x streams on scalar so
            # tile 0 lands as early as possible.
            # x arrives in bursts balanced over both HWDGE queues, in
            # consumption order; wqkv lands in thirds (V first) between the
            # early bursts. Burst sizing trades per-DMA completion latency
            # (~2us) against getting tile 0 started early.
            def _xburst(eng, lo, hi):
                lo = min(lo, NT)
                hi = min(hi, NT)
                if lo < hi:
                    eng.dma_start(out=x_tiles[:, lo:hi, :],
                                  in_=x_r[:, lo:hi, :])
            def _wpiece(eng, w):
                eng.dma_start(out=wqkv_sb[:, :, w * D:(w + 1) * D],
                              in_=wqkv_d[:, :, w * D:(w + 1) * D])
            # ALL of x on the sync ring in strict consumption order: the
            # ring's FIFO sequences the bursts deterministically (no cross-
            # queue HBM races for the pipeline-critical head), and the scalar
            # engine issues no DMAs at all so ACT work is never blocked.
            _wpiece(nc.gpsimd, 2)   # W_v first on the gpsimd ring: it
                                    # gates V(0); bqkv isn't needed until the
                                    # first k-copy much later
            nc.gpsimd.dma_start(out=bqkv_sb, in_=bqkv_d[:])
            _xburst(nc.sync, 0, 2)
            _xburst(nc.sync, 2, 6)
            _wpiece(nc.sync, 1)     # W_k
            _xburst(nc.sync, 6, 10)
            _wpiece(nc.sync, 0)     # W_q
            _xburst(nc.sync, 10, NT)
            nc.gpsimd.dma_start(out=wfc_sb, in_=wfc_d[:])
            bfc_bcast = bass.AP(
                tensor=bfc_d.ap().tensor, offset=0,
                ap=[[0, 128]] + bfc_d.ap().ap[1:])
            nc.gpsimd.dma_start(out=bfc_sb, in_=bfc_bcast)

            def emit_score_pairs(sc, E, esum, tp_lo, tp_hi):
                # scores + exp; the softmax denominators accumulate on DVE
                # (esum[p,s] = sum_tt E[tt*128+p, s]) so the PE only pays one
                # f32r ones-matmul per chunk for the cross-partition sum.
                for tp in range(tp_lo, tp_hi):
                    ps = psA.tile([128, 2, 512], F32, tag="mm", name="pss")
                    for half in range(2):
                        tt = 2 * tp + half
                        for et in range(ND):
                            nc.tensor.matmul(
                                ps[:, half, :],
                                kT[:, et, tt * 128:(tt + 1) * 128],
                                qT[:, et, sc * 512:(sc + 1) * 512],
                                start=(et == 0), stop=(et == ND - 1),
                            )
                    nc.scalar.activation(
                        out=E[:, 2 * tp:2 * tp + 2, :], in_=ps, func=AF.Exp,
                        bias=zero_sb, scale=SM_SCALE,
                    )
                    if tp == tp_lo == 0:
                        nc.vector.tensor_copy(out=esum, in_=E[:, 0, :])
                        nc.vector.tensor_add(out=esum, in0=esum,
                                             in1=E[:, 1, :])
                    else:
                        for half in range(2):
                            nc.vector.tensor_add(
                                out=esum, in0=esum,
                                in1=E[:, 2 * tp + half, :])

            xn_insts = []
            cp_insts = []
            for it in range(NT):
                stat = lnp.tile([128, 6], F32, tag="stat")
                bn_inst = nc.vector.bn_stats(out=stat, in_=x_tiles[:, it, :])
                if it >= 2:
                    # keep the DVE stream interleaved: without this edge the
                    # scheduler front-loads all (DMA-paced) bn_stats and the
                    # normalize chain head-of-line blocks behind them
                    add_dep_helper(bn_inst.ins, xn_insts[it - 2].ins, sync=False,
                                   reason="interleave LN chain")
                mv = lnp.tile([128, 2], F32, tag="mv")
                nc.vector.bn_aggr(out=mv, in_=stat)
                stdv = lnp.tile([128, 1], F32, tag="stdv")
                rstd = lnp.tile([128, 1], F32, tag="rstd")
                xn = lnp.tile([128, D], BF16, tag="xn")
                # sqrt at NORMAL priority: boosting it sorts all (DMA-gated)
                # sqrts ahead of the ready xnT copies in the ACT stream and
                # head-of-line blocks them
                nc.scalar.activation(out=stdv, in_=mv[:, 1:2],
                                     func=AF.Sqrt, bias=eps_sb)
                # high priority: don't let later (DMA-paced) bn_stats get
                # ahead of the normalize chain in the in-order DVE stream
                with tc.high_priority():
                    nc.vector.reciprocal_approx_fast(out=rstd, in_=stdv)
                    xn_insts.append(nc.vector.tensor_scalar(
                        out=xn, in0=x_tiles[:, it, :], scalar1=mv[:, 0:1],
                        scalar2=rstd, op0=OP.subtract, op1=OP.mult,
                    ))
                # transpose via regular N=128 bf16 matmul against identity;
                # lands in the (phase-A-idle) FC psum pool to widen rotation
                pt = psFC.tile([128, ND, 128], F32, tag="fc", name="pt")
                for j in range(ND):
                    nc.tensor.matmul(
                        pt[:, j, :],
                        xn[:, j * 128:(j + 1) * 128],
                        ident,
                        start=True, stop=True,
                    )
                cp_insts.append(nc.scalar.activation(
                    out=xnT[:, :, it * 128:(it + 1) * 128], in_=pt,
                    func=AF.Identity, bias=zero_sb,
                ))
                # V row-tile: ready as soon as this xnT tile lands
                ps = psA.tile([128, 512], F32, tag="mm", name="psv")
                for dt in range(ND):
                    nc.tensor.matmul(
                        ps,
                        xnT[:, dt, it * 128:(it + 1) * 128],
                        wqkv_sb[:, dt, 2 * D:3 * D],
                        start=(dt == 0), stop=(dt == ND - 1),
                    )
                nc.scalar.activation(out=vv[:, it, :], in_=ps,
                                     func=AF.Identity, bias=zero_sb)

                # after each group of 4 tiles, the matching kT/qT chunk;
                # k copies ride ACT (Identity + per-partition bias), q stays
                # on DVE -- balances the two engines through phase A.
                if it % 4 == 3:
                    sc = it // 4
                    for et in range(ND):
                        ps = psA.tile([128, 512], F32, tag="mm", name="psk")
                        for dt in range(ND):
                            nc.tensor.matmul(
                                ps,
                                wqkv_sb[:, dt, D + et * 128: D + (et + 1) * 128],
                                xnT[:, dt, sc * 512:(sc + 1) * 512],
                                start=(dt == 0), stop=(dt == ND - 1),
                            )
                        nc.scalar.activation(
                            out=kT[:, et, sc * 512:(sc + 1) * 512], in_=ps,
                            func=AF.Identity,
                            bias=bqkv_sb[:, ND + et: ND + et + 1],
                        )
                    for et in range(ND):
                        ps = psA.tile([128, 512], F32, tag="mm", name="psq")
                        for dt in range(ND):
                            nc.tensor.matmul(
                                ps,
                                wqkv_sb[:, dt, et * 128:(et + 1) * 128],
                                xnT[:, dt, sc * 512:(sc + 1) * 512],
                                start=(dt == 0), stop=(dt == ND - 1),
                            )
                        nc.vector.tensor_scalar_add(
                            out=qT[:, et, sc * 512:(sc + 1) * 512], in0=ps,
                            scalar1=bqkv_sb[:, et:et + 1],
                        )
                    # overlap chunk-0 attention with the rest of phase A:
                    # its score pairs only need qT[0] + the kT tiles done so far
                    if NSC > 1:
                        if sc == 0:
                            E0 = ebp.tile([128, NT, 512], BF16, tag="E",
                                          name="E0")
                            es0 = esb.tile([128, 512], F32R, tag="es",
                                           name="es0")
                            c0_done = 0
                        else:
                            hi = min((it + 1) // 2, NT // 2)
                            emit_score_pairs(0, E0, es0, c0_done, hi)
                            c0_done = hi

            # ---- phase C: attention + FC, per query chunk ----
            for sc in range(NSC):
                if NSC > 1 and sc == 0:
                    E = E0
                    esum = es0
                    emit_score_pairs(0, E, esum, c0_done, NT // 2)
                else:
                    E = ebp.tile([128, NT, 512], BF16, tag="E")
                    esum = esb.tile([128, 512], F32R, tag="es", name="esum")
                    emit_score_pairs(sc, E, esum, 0, NT // 2)
                zinv = zbp.tile([128, 512], F32, tag="zinv")
                oT = ebp.tile([128, ND, 512], BF16, tag="oT")
                zp = None
                for dt in range(ND):
                    op = psO.tile([128, 512], F32, tag="o", name=f"op{dt}")
                    for tt in range(NT):
                        nc.tensor.matmul(
                            op,
                            vv[:, tt, dt * 128:(dt + 1) * 128],
                            E[:, tt, :],
                            start=(tt == 0), stop=(tt == NT - 1),
                        )
                    if dt == 0:
                        # Z after the first PV pass: PV needs only E, so the
                        # PE isn't stalled waiting for the DVE esum tail
                        zp = psFC.tile([128, 512], F32, tag="fc", name="zp")
                        nc.tensor.matmul(zp, ones_r, esum,
                                         start=True, stop=True)
                        nc.vector.reciprocal_approx_fast(out=zinv, in_=zp)
                    nc.vector.tensor_mul(out=oT[:, dt, :], in0=op, in1=zinv)
                    if has_bv:
                        nc.vector.tensor_scalar_add(
                            out=oT[:, dt, :], in0=oT[:, dt, :],
                            scalar1=bqkv_sb[:, 2 * ND + dt: 2 * ND + dt + 1],
                        )

                for ss in range(4):
                    ps = psFC.tile([128, 512], F32, tag="fc")
                    for dt in range(ND):
                        nc.tensor.matmul(
                            ps,
                            oT[:, dt, ss * 128:(ss + 1) * 128],
                            wfc_sb[:, dt, :],
                            start=(dt == 0), stop=(dt == ND - 1),
                        )
                    # y = ps + b_fc (broadcast rows), LeakyReLU = (y*slope) max y
                    yb = ybp.tile([128, D], F32, tag="yb")
                    nc.vector.tensor_add(out=yb, in0=ps, in1=bfc_sb)
                    yt = ybp.tile([128, D], F32, tag="y")
                    nc.vector.scalar_tensor_tensor(
                        out=yt, in0=yb, scalar=SLOPE, in1=yb,
                        op0=OP.mult, op1=OP.max,
                    )
                    r0 = sc * 512 + ss * 128
                    nc.sync.dma_start(out=out_d[r0:r0 + 128, :], in_=yt)

    nc.compile()
    return nc


_NC_CACHE = {}


def _get_nc(S, has_bv):
    key = (S, has_bv)
    if key not in _NC_CACHE:
        _NC_CACHE[key] = build_nc(S, has_bv)
    return _NC_CACHE[key]


def prep_inputs(x, ln_gamma, ln_beta, W_qkv, W_fc, b_fc):
    bf = ml_dtypes.bfloat16
    W_qkv = np.asarray(W_qkv, dtype=np.float32)
    Wq = W_qkv * np.asarray(ln_gamma, dtype=np.float32)[:, None]
    wqkv_t = np.ascontiguousarray(
        Wq.reshape(ND, 128, E3).transpose(1, 0, 2)).astype(bf)
    wfc_t = np.ascontiguousarray(
        np.asarray(W_fc, dtype=np.float32).reshape(ND, 128, D).transpose(1, 0, 2)
    ).astype(bf)
    bqkv = np.asarray(ln_beta, dtype=np.float32) @ W_qkv  # [1536]
    bqkv_t = np.ascontiguousarray(bqkv.reshape(3 * ND, 128).T)
    bfc_t = np.asarray(b_fc, dtype=np.float32).reshape(1, D)
    has_bv = bool(np.any(bqkv[2 * D:]))
    return wqkv_t, wfc_t, bqkv_t, bfc_t, has_bv


def run(x, ln_gamma, ln_beta, W_qkv, W_fc, b_fc, trace=False):
    x = np.asarray(x, dtype=np.float32)
    B, S, Din = x.shape
    assert B == N_CORES and Din == D and S % 512 == 0, (B, S, Din)
    wqkv_t, wfc_t, bqkv_t, bfc_t, has_bv = prep_inputs(
        x, ln_gamma, ln_beta, W_qkv, W_fc, b_fc)
    nc = _get_nc(S, has_bv)
    in_maps = [
        {
            "x": np.ascontiguousarray(x[b]),
            "wqkv": wqkv_t,
            "wfc": wfc_t,
            "bqkv": bqkv_t,
            "bfc": bfc_t,
        }
        for b in range(B)
    ]
    res = run_bass_kernel_spmd(nc, in_maps, core_ids=list(range(B)), trace=trace)
    out = np.stack([res.results[b]["out"] for b in range(B)]).astype(np.float32)
    return out, res


def kernel(x, ln_gamma, ln_beta, W_qkv, W_fc, b_fc):
    out, _ = run(x, ln_gamma, ln_beta, W_qkv, W_fc, b_fc)
    return out
